# revision 12
# baseline (speedup 1.0000x reference)
"""Trainium2 Bass kernel for Luong bilinear attention.

  out = softmax((q @ w) @ k^T) @ v      q:[B,Lq,Din] k,v:[B,Lk,Dout] w:[Din,Dout]

Sharding: 8 cores = 4 batches x 2 halves of Lq (data-parallel over batch,
sequence-parallel over Lq). k, v are replicated across the 2 cores of a batch.

Per-core layout strategy: scores are computed transposed, sT[k, q], so the
softmax denominator and the attention*V product are both plain matmuls with
k as the contraction (partition) dim:
    wqT[o, q] = w[i,o]^T . qT[i, q]          (PE, fp16)
    sT[k, q]  = kT[o, k]^T . wqT[o, q]       (PE, fp16, f32 PSUM)
    p[k, q]   = exp(sT)                      (ScalarE, f32 -> bf16)
    acc[q, 0:257] = p^T . [v | ones]         (PE; col 256 = softmax denom)
    out[q, o] = acc[:, 0:256] * (1/acc[:, 256])   (DVE)
exp() is applied without max-subtraction: scores ~ N(0, 12.8), |s| < ~70,
exp stays comfortably inside f32/bf16 range, and softmax is shift-invariant.

The kernel is PE-issue-bound in steady state, so the remaining time is at
the seams. Cold-start schedule:
  - PE warm-up matmuls read a memset dummy tile (no DMA dependency) so the
    clock ramps from the entry barrier while input DMAs are in flight.
  - All input DMAs are issued on one FIFO ring (sync) in the priority order
    compute consumes them: w, then all qT pieces (wq runs back-to-back with
    the warm-up while the ring moves on to kT), then kT pieces (which pace
    scores(0) but arrive ~2x faster than PE consumes them), v last. The PE
    queue is in-order, so ring order must match PE consumption order.
  - Inputs are marshalled on the host into piece-major blocks so each DMA
    piece reads 2-4KB contiguous runs per partition (large DMA packets).
"""

import numpy as np

B, LQ, LK, DIN, DOUT = 4, 4096, 4096, 256, 256
N_CORES = 8
QS = LQ // (N_CORES // B)  # 2048 queries per core
QC = 512                   # q-chunk (matmul free dim)
NQC = QS // QC             # 4 chunks
NKT = LK // 128            # 32 k tiles
VN = DOUT + 1              # v plus ones column
KPC = 8                    # kT load pieces
KP = LK // KPC             # 512 keys per piece
VPC = 4                    # v load pieces
VKT = NKT // VPC           # 8 k-tiles per v piece

_prog_cache: dict = {}


def build_program(repeat: int = 1):
    """Build the (SPMD-identical) per-core Bass program."""
    if repeat in _prog_cache:
        return _prog_cache[repeat]
    from contextlib import ExitStack

    import concourse.bacc as bacc
    import concourse.mybir as mybir
    import concourse.tile as tile

    BF16 = mybir.dt.bfloat16
    FP16 = mybir.dt.float16
    F32 = mybir.dt.float32
    EXP = mybir.ActivationFunctionType.Exp

    nc = bacc.Bacc(
        "TRN2", target_bir_lowering=False, debug=False, num_devices=N_CORES
    )
    # piece-major host-marshalled layouts (partition dim second, so each
    # (piece, partition) run is contiguous in DRAM)
    qT_d = nc.dram_tensor("qT", [NQC, 128, 2, QC], FP16, kind="ExternalInput")
    kT_d = nc.dram_tensor("kT", [KPC, 128, 2, KP], FP16, kind="ExternalInput")
    v_d = nc.dram_tensor("v", [VPC, 128, VKT, DOUT], BF16, kind="ExternalInput")
    w_d = nc.dram_tensor("w", [128, 2, DOUT], FP16, kind="ExternalInput")
    o_d = nc.dram_tensor("o", [QS, DOUT], F32, kind="ExternalOutput")

    with tile.TileContext(nc) as tc, ExitStack() as ctx:
        persist = ctx.enter_context(tc.tile_pool(name="persist", bufs=1))
        pexp = ctx.enter_context(tc.tile_pool(name="pexp", bufs=2))
        ps_pool = ctx.enter_context(
            tc.tile_pool(name="ps", bufs=3, space="PSUM")
        )
        po_pool = ctx.enter_context(
            tc.tile_pool(name="po", bufs=2, space="PSUM")
        )
        outp = ctx.enter_context(tc.tile_pool(name="outp", bufs=4))

        NWARM = 20
        for _ in range(repeat):
            # ---- PE warm-up on a memset dummy: no DMA dependency, so the
            # ---- array is busy from the entry barrier and the clock ramps
            # ---- while the first input DMAs are still in flight
            warm = persist.tile([128, 256], FP16, tag="warm")
            nc.vector.memset(warm[:], 1.0)
            wps = ps_pool.tile([128, 2, QC], F32, tag="ps")
            for _i in range(NWARM):
                nc.tensor.matmul(
                    wps[:, 0, 0:256], warm[:, 0:128], warm[:],
                    start=True, stop=True,
                )

            # ---- input loads: one FIFO ring (sync), in consumption order.
            # ---- A single ring beats spreading across engine queues: the
            # ---- gpsimd queue generates descriptors in software (slow) and
            # ---- cross-queue arbitration delays the first-needed pieces.
            w_bf = persist.tile([128, 2, DOUT], FP16, tag="w_bf")
            qT_bf = persist.tile([128, 2, QS], FP16, tag="qT_bf")
            kT_bf = persist.tile([128, 2, LK], FP16, tag="kT_bf")
            v_bf = persist.tile([128, NKT, VN], BF16, tag="v_bf")
            nc.vector.memset(v_bf[:, :, DOUT : DOUT + 1], 1.0)

            def load_q(qc, eng):
                sl = slice(qc * QC, (qc + 1) * QC)
                eng.dma_start(qT_bf[:, :, sl], qT_d.ap()[qc])

            def load_kp(kp, eng):
                sl = slice(kp * KP, (kp + 1) * KP)
                eng.dma_start(kT_bf[:, :, sl], kT_d.ap()[kp])

            def load_v(vh):
                sl = slice(vh * VKT, (vh + 1) * VKT)
                nc.sync.dma_start(v_bf[:, sl, 0:DOUT], v_d.ap()[vh])

            nc.sync.dma_start(w_bf[:], w_d.ap())
            for qc in range(NQC):
                load_q(qc, nc.sync)
            for kp in range(KPC):
                load_kp(kp, nc.sync)
            for vh in range(VPC):
                load_v(vh)

            # ---- wqT[o, q] = w^T . qT, one qc chunk at a time. Each chunk
            # ---- is emitted just before the scores() phase that consumes
            # ---- it (see main loop) so a late qT piece can never block
            # ---- already-runnable scores work in the in-order PE queue.
            wq_bf = persist.tile([128, 2, QS], FP16, tag="wq_bf")

            def wq(qc):
                ps = ps_pool.tile([128, 2, QC], F32, tag="ps")
                for ot in range(2):
                    for it in range(2):
                        nc.tensor.matmul(
                            ps[:, ot, :],
                            w_bf[:, it, ot * 128 : (ot + 1) * 128],
                            qT_bf[:, it, qc * QC : (qc + 1) * QC],
                            start=(it == 0),
                            stop=(it == 1),
                        )
                nc.vector.tensor_copy(
                    wq_bf[:, :, qc * QC : (qc + 1) * QC], ps[:, :, :]
                )

            # ---- main loop: emit scores(qc+1) before AV(qc) so ScalarE's
            # ---- exp always has PE runway to hide behind
            def scores(qc):
                p_all = pexp.tile([128, NKT, QC], BF16, tag="p_all")
                for ktg in range(NKT // 2):
                    ps = ps_pool.tile([128, 2, QC], F32, tag="ps")
                    for j in range(2):
                        kt = ktg * 2 + j
                        for it in range(2):
                            nc.tensor.matmul(
                                ps[:, j, :],
                                kT_bf[:, it, kt * 128 : (kt + 1) * 128],
                                wq_bf[:, it, qc * QC : (qc + 1) * QC],
                                start=(it == 0),
                                stop=(it == 1),
                            )
                    nc.scalar.activation(
                        p_all[:, ktg * 2 : (ktg + 1) * 2, :], ps[:, :, :], EXP
                    )
                return p_all

            def av(qc, p_all):
                for qt in range(QC // 128):
                    po = po_pool.tile([128, VN], F32, tag="po")
                    for kt in range(NKT):
                        nc.tensor.matmul(
                            po[:],
                            p_all[:, kt, qt * 128 : (qt + 1) * 128],
                            v_bf[:, kt, :],
                            start=(kt == 0),
                            stop=(kt == NKT - 1),
                        )
                    rec = outp.tile([128, 1], F32, tag="rec")
                    nc.vector.reciprocal(rec[:], po[:, DOUT : DOUT + 1])
                    o_sb = outp.tile([128, DOUT], F32, tag="o_sb")
                    nc.vector.tensor_scalar_mul(o_sb[:], po[:, 0:DOUT], rec[:])
                    r0 = (qc * (QC // 128) + qt) * 128
                    nc.sync.dma_start(o_d.ap()[r0 : r0 + 128, :], o_sb[:])

            wq(0)
            p_prev = scores(0)
            for qc in range(1, NQC):
                wq(qc)
                p_cur = scores(qc)
                av(qc - 1, p_prev)
                p_prev = p_cur
            av(NQC - 1, p_prev)

    nc.compile()
    _prog_cache[repeat] = nc
    return nc


def make_in_maps(q, k, v, w):
    """Shard + marshal full inputs into per-core input maps.

    Marshalling includes the transpose of q/k, the rounding to the kernel's
    compute dtypes (fp16 score path, bf16 values), and the piece-major
    reblocking that gives the device DMA engine large contiguous reads.
    """
    import ml_dtypes

    q = np.asarray(q, dtype=np.float32)
    k = np.asarray(k, dtype=np.float32)
    v = np.asarray(v, dtype=np.float32)

    # w[i, o] -> [p, t, o] with i = t*128 + p
    w16 = (
        np.asarray(w, dtype=np.float32)
        .astype(np.float16)
        .reshape(2, 128, DOUT)
        .transpose(1, 0, 2)
        .copy()
    )
    # k[b][l, o] -> kT[o, l] -> [kp, p, t, kq] with o = t*128 + p, l = kp*KP + kq
    kT = []
    for b in range(B):
        kb = k[b].T.astype(np.float16)            # [DOUT, LK]
        kb = kb.reshape(2, 128, KPC, KP)          # [t, p, kp, kq]
        kT.append(kb.transpose(2, 1, 0, 3).copy())  # [kp, p, t, kq]
    # v[b][l, o] -> [vh, p, ktl, o] with l = (vh*VKT + ktl)*128 + p
    vb = []
    for b in range(B):
        x = v[b].astype(ml_dtypes.bfloat16)       # [LK, DOUT]
        x = x.reshape(VPC, VKT, 128, DOUT)        # [vh, ktl, p, o]
        vb.append(x.transpose(0, 2, 1, 3).copy())  # [vh, p, ktl, o]

    in_maps = []
    for c in range(N_CORES):
        b, h = divmod(c, N_CORES // B)
        # q[b][l, i] -> qT[i, lq] -> [qc, p, t, ql] with i = t*128+p
        qb = q[b, h * QS : (h + 1) * QS, :].T.astype(np.float16)  # [DIN, QS]
        qb = qb.reshape(2, 128, NQC, QC)          # [t, p, qc, ql]
        qb = qb.transpose(2, 1, 0, 3).copy()      # [qc, p, t, ql]
        in_maps.append({"qT": qb, "kT": kT[b], "v": vb[b], "w": w16})
    return in_maps


def kernel(q, v, k, w):
    from concourse import bass_utils

    nc = build_program()
    in_maps = make_in_maps(q, k, v, w)
    res = bass_utils.run_bass_kernel_spmd(nc, in_maps, core_ids=list(range(N_CORES)))
    out = np.empty((B, LQ, DOUT), dtype=np.float32)
    for c in range(N_CORES):
        b, h = divmod(c, N_CORES // B)
        out[b, h * QS : (h + 1) * QS, :] = res.results[c]["o"]
    return out


# revision 13
# speedup vs baseline: 1.1564x; 1.1564x over previous
"""Trainium2 Bass kernel for Luong bilinear attention.

  out = softmax((q @ w) @ k^T) @ v      q:[B,Lq,Din] k,v:[B,Lk,Dout] w:[Din,Dout]

Sharding: 8 cores = 4 batches x 2 halves of Lq (data-parallel over batch,
sequence-parallel over Lq). k, v are replicated across the 2 cores of a batch.

Per-core layout strategy: scores are computed transposed, sT[k, q], so the
softmax denominator and the attention*V product are both plain matmuls with
k as the contraction (partition) dim:
    wqT[o, q] = w[i,o]^T . qT[i, q]          (PE, fp16)
    sT[k, q]  = kT[o, k]^T . wqT[o, q]       (PE, fp16, f32 PSUM)
    p[k, q]   = exp(sT)                      (ScalarE, f32 -> bf16)
    acc[q, 0:257] = p^T . [v | ones]         (PE; col 256 = softmax denom)
    out[q, o] = acc[:, 0:256] * (1/acc[:, 256])   (DVE)
exp() is applied without max-subtraction: scores ~ N(0, 12.8), |s| < ~70,
exp stays comfortably inside f32/bf16 range, and softmax is shift-invariant.

The kernel is PE-issue-bound in steady state, so the remaining time is at
the seams. Cold-start schedule:
  - PE warm-up matmuls read a memset dummy tile (no DMA dependency) so the
    clock ramps from the entry barrier while input DMAs are in flight.
  - All input DMAs are issued on one FIFO ring (sync) in the priority order
    compute consumes them: w, then all qT pieces (wq runs back-to-back with
    the warm-up while the ring moves on to kT), then kT pieces (which pace
    scores(0) but arrive ~2x faster than PE consumes them), v last. The PE
    queue is in-order, so ring order must match PE consumption order.
  - Inputs are marshalled on the host into piece-major blocks so each DMA
    piece reads 2-4KB contiguous runs per partition (large DMA packets).
"""

import numpy as np

B, LQ, LK, DIN, DOUT = 4, 4096, 4096, 256, 256
N_CORES = 8
QS = LQ // (N_CORES // B)  # 2048 queries per core
QC = 512                   # q-chunk (matmul free dim)
NQC = QS // QC             # 4 chunks
NKT = LK // 128            # 32 k tiles
VN = DOUT + 1              # v plus ones column
KPC = 8                    # kT load pieces
KP = LK // KPC             # 512 keys per piece
VPC = 4                    # v load pieces
VKT = NKT // VPC           # 8 k-tiles per v piece

_prog_cache: dict = {}


def build_program(repeat: int = 1):
    """Build the (SPMD-identical) per-core Bass program."""
    if repeat in _prog_cache:
        return _prog_cache[repeat]
    from contextlib import ExitStack

    import concourse.bacc as bacc
    import concourse.mybir as mybir
    import concourse.tile as tile

    BF16 = mybir.dt.bfloat16
    FP16 = mybir.dt.float16
    F32 = mybir.dt.float32
    EXP = mybir.ActivationFunctionType.Exp

    nc = bacc.Bacc(
        "TRN2", target_bir_lowering=False, debug=False, num_devices=N_CORES
    )
    # piece-major host-marshalled layouts (partition dim second, so each
    # (piece, partition) run is contiguous in DRAM)
    qT_d = nc.dram_tensor("qT", [NQC, 128, 2, QC], FP16, kind="ExternalInput")
    kT_d = nc.dram_tensor("kT", [KPC, 128, 2, KP], FP16, kind="ExternalInput")
    v_d = nc.dram_tensor("v", [VPC, 128, VKT, DOUT], BF16, kind="ExternalInput")
    w_d = nc.dram_tensor("w", [128, 2, DOUT], FP16, kind="ExternalInput")
    o_d = nc.dram_tensor("o", [QS, DOUT], F32, kind="ExternalOutput")

    with tile.TileContext(nc) as tc, ExitStack() as ctx:
        persist = ctx.enter_context(tc.tile_pool(name="persist", bufs=1))
        pexp = ctx.enter_context(tc.tile_pool(name="pexp", bufs=2))
        ps_pool = ctx.enter_context(
            tc.tile_pool(name="ps", bufs=3, space="PSUM")
        )
        po_pool = ctx.enter_context(
            tc.tile_pool(name="po", bufs=2, space="PSUM")
        )
        outp = ctx.enter_context(tc.tile_pool(name="outp", bufs=4))

        NWARM = 20
        for _ in range(repeat):
            # ---- PE warm-up on a memset dummy: no DMA dependency, so the
            # ---- array is busy from the entry barrier and the clock ramps
            # ---- while the first input DMAs are still in flight
            warm = persist.tile([128, 256], FP16, tag="warm")
            nc.vector.memset(warm[:], 1.0)
            wps = ps_pool.tile([128, 2, QC], F32, tag="ps")
            for _i in range(NWARM):
                nc.tensor.matmul(
                    wps[:, 0, 0:256], warm[:, 0:128], warm[:],
                    start=True, stop=True,
                )

            # ---- input loads: one FIFO ring (sync), in consumption order.
            # ---- A single ring beats spreading across engine queues: the
            # ---- gpsimd queue generates descriptors in software (slow) and
            # ---- cross-queue arbitration delays the first-needed pieces.
            w_bf = persist.tile([128, 2, DOUT], FP16, tag="w_bf")
            qT_bf = persist.tile([128, 2, QS], FP16, tag="qT_bf")
            kT_bf = persist.tile([128, 2, LK], FP16, tag="kT_bf")
            v_bf = persist.tile([128, NKT, VN], BF16, tag="v_bf")
            nc.vector.memset(v_bf[:, :, DOUT : DOUT + 1], 1.0)

            def load_q(qc, eng):
                sl = slice(qc * QC, (qc + 1) * QC)
                eng.dma_start(qT_bf[:, :, sl], qT_d.ap()[qc])

            def load_kp(kp, eng):
                sl = slice(kp * KP, (kp + 1) * KP)
                eng.dma_start(kT_bf[:, :, sl], kT_d.ap()[kp])

            def load_v(vh):
                sl = slice(vh * VKT, (vh + 1) * VKT)
                nc.sync.dma_start(v_bf[:, sl, 0:DOUT], v_d.ap()[vh])

            nc.sync.dma_start(w_bf[:], w_d.ap())
            for qc in range(NQC):
                load_q(qc, nc.sync)
            for kp in range(KPC):
                load_kp(kp, nc.sync)
            for vh in range(VPC):
                load_v(vh)

            # ---- wqT[o, q] = w^T . qT, one qc chunk at a time. Each chunk
            # ---- is emitted just before the scores() phase that consumes
            # ---- it (see main loop) so a late qT piece can never block
            # ---- already-runnable scores work in the in-order PE queue.
            wq_bf = persist.tile([128, 2, QS], FP16, tag="wq_bf")

            def wq(qc):
                ps = ps_pool.tile([128, 2, QC], F32, tag="ps")
                for ot in range(2):
                    for it in range(2):
                        nc.tensor.matmul(
                            ps[:, ot, :],
                            w_bf[:, it, ot * 128 : (ot + 1) * 128],
                            qT_bf[:, it, qc * QC : (qc + 1) * QC],
                            start=(it == 0),
                            stop=(it == 1),
                        )
                    # per-half copy: scores(qc)'s first (it=0) matmuls only
                    # wait for the ot=0 half, which copies while the ot=1
                    # matmuls still run
                    nc.vector.tensor_copy(
                        wq_bf[:, ot, qc * QC : (qc + 1) * QC], ps[:, ot, :]
                    )

            # ---- main loop: emit scores(qc+1) before AV(qc) so ScalarE's
            # ---- exp always has PE runway to hide behind
            def scores(qc):
                p_all = pexp.tile([128, NKT, QC], BF16, tag="p_all")
                for ktg in range(NKT // 2):
                    ps = ps_pool.tile([128, 2, QC], F32, tag="ps")
                    for j in range(2):
                        kt = ktg * 2 + j
                        for it in range(2):
                            nc.tensor.matmul(
                                ps[:, j, :],
                                kT_bf[:, it, kt * 128 : (kt + 1) * 128],
                                wq_bf[:, it, qc * QC : (qc + 1) * QC],
                                start=(it == 0),
                                stop=(it == 1),
                            )
                    nc.scalar.activation(
                        p_all[:, ktg * 2 : (ktg + 1) * 2, :], ps[:, :, :], EXP
                    )
                return p_all

            def av(qc, p_all):
                for qt in range(QC // 128):
                    po = po_pool.tile([128, VN], F32, tag="po")
                    for kt in range(NKT):
                        nc.tensor.matmul(
                            po[:],
                            p_all[:, kt, qt * 128 : (qt + 1) * 128],
                            v_bf[:, kt, :],
                            start=(kt == 0),
                            stop=(kt == NKT - 1),
                        )
                    rec = outp.tile([128, 1], F32, tag="rec")
                    nc.vector.reciprocal(rec[:], po[:, DOUT : DOUT + 1])
                    o_sb = outp.tile([128, DOUT], F32, tag="o_sb")
                    nc.vector.tensor_scalar_mul(o_sb[:], po[:, 0:DOUT], rec[:])
                    r0 = (qc * (QC // 128) + qt) * 128
                    nc.sync.dma_start(o_d.ap()[r0 : r0 + 128, :], o_sb[:])

            wq(0)
            p_prev = scores(0)
            for qc in range(1, NQC):
                wq(qc)
                p_cur = scores(qc)
                av(qc - 1, p_prev)
                p_prev = p_cur
            av(NQC - 1, p_prev)

    nc.compile()
    _prog_cache[repeat] = nc
    return nc


def make_in_maps(q, k, v, w):
    """Shard + marshal full inputs into per-core input maps.

    Marshalling includes the transpose of q/k, the rounding to the kernel's
    compute dtypes (fp16 score path, bf16 values), and the piece-major
    reblocking that gives the device DMA engine large contiguous reads.
    """
    import ml_dtypes

    q = np.asarray(q, dtype=np.float32)
    k = np.asarray(k, dtype=np.float32)
    v = np.asarray(v, dtype=np.float32)

    # w[i, o] -> [p, t, o] with i = t*128 + p
    w16 = (
        np.asarray(w, dtype=np.float32)
        .astype(np.float16)
        .reshape(2, 128, DOUT)
        .transpose(1, 0, 2)
        .copy()
    )
    # k[b][l, o] -> kT[o, l] -> [kp, p, t, kq] with o = t*128 + p, l = kp*KP + kq
    kT = []
    for b in range(B):
        kb = k[b].T.astype(np.float16)            # [DOUT, LK]
        kb = kb.reshape(2, 128, KPC, KP)          # [t, p, kp, kq]
        kT.append(kb.transpose(2, 1, 0, 3).copy())  # [kp, p, t, kq]
    # v[b][l, o] -> [vh, p, ktl, o] with l = (vh*VKT + ktl)*128 + p
    vb = []
    for b in range(B):
        x = v[b].astype(ml_dtypes.bfloat16)       # [LK, DOUT]
        x = x.reshape(VPC, VKT, 128, DOUT)        # [vh, ktl, p, o]
        vb.append(x.transpose(0, 2, 1, 3).copy())  # [vh, p, ktl, o]

    in_maps = []
    for c in range(N_CORES):
        b, h = divmod(c, N_CORES // B)
        # q[b][l, i] -> qT[i, lq] -> [qc, p, t, ql] with i = t*128+p
        qb = q[b, h * QS : (h + 1) * QS, :].T.astype(np.float16)  # [DIN, QS]
        qb = qb.reshape(2, 128, NQC, QC)          # [t, p, qc, ql]
        qb = qb.transpose(2, 1, 0, 3).copy()      # [qc, p, t, ql]
        in_maps.append({"qT": qb, "kT": kT[b], "v": vb[b], "w": w16})
    return in_maps


def kernel(q, v, k, w):
    from concourse import bass_utils

    nc = build_program()
    in_maps = make_in_maps(q, k, v, w)
    res = bass_utils.run_bass_kernel_spmd(nc, in_maps, core_ids=list(range(N_CORES)))
    out = np.empty((B, LQ, DOUT), dtype=np.float32)
    for c in range(N_CORES):
        b, h = divmod(c, N_CORES // B)
        out[b, h * QS : (h + 1) * QS, :] = res.results[c]["o"]
    return out
